# revision 2
# baseline (speedup 1.0000x reference)
"""NMI loss (soft-histogram mutual information) on 8 trn2 cores.

Voxel-sharded: each core handles N/8 = 262144 voxels as [128, 2048].
Per chunk of Vc=512 voxel-columns it builds the dense 34-bin Gaussian
window matrix (32 real bins + 2 "virtual" bins at centers -1/31 and
32/31) in fp16, i-major layout [P, bin_block, Vc]:

  d_i   = x - c_i          DVE tensor_scalar (fp16, 4x) for bins >= NACT,
                           ACT Square(x + bias) for bins < NACT (fused)
  d_i^2 = d_i * d_i        DVE in-place self-mult (fp16, 2x)
  I_i   = exp(-p d_i^2)    ACT big-block Exp

The per-voxel normalizer S = sum_i I_i is computed analytically from the
Gaussian-comb identity: S = sqrt(pi/2) - I[-1] - I[32] (the two virtual
bins are exactly the edge-deficit terms; the 2q*cos harmonic contributes
only ~1e-4 relative bias to the final MI and is dropped). 1/S comes from
ACT as exp(-ln(S)) (Ln and Exp share one activation table set).

One 33x33 Gram accumulates on the TensorEngine over all voxel-columns:
  lhsT = [I_a/S_a | 1/S_b], rhs = [I_b | 1]
giving N*pab, N*pa (col 32), N*pb (row 32). The 8 partial stats go to
the host, which sums them and does the tiny log-MI reduction exactly as
the reference.

Raw Bass blocks with manual semaphores; buffers are depth-2 over chunks.
fp16 tensor_scalar requires f32 AP scalars (immediates mis-encode).
"""

import sys
import numpy as np

sys.path.insert(0, "/opt/trn_rl_repo")

NCORES = 8
P = 128
B = 32                     # histogram bins
S = B + 1                  # Gram size (bins + marginal slot)
NBLK = 35                  # buffer blocks: 0..31 bins, 32 extra, 33/34 virtual
NVOX_TOTAL = 128 ** 3
NVOX = NVOX_TOTAL // NCORES
V = NVOX // P              # 2048 voxel-columns per partition
NCHUNK = 4
VC = V // NCHUNK           # 512
NACT = 4                   # bins whose (x-c)^2 is fused on ACT via Square+bias

# replicate reference's f32 constant computation
_BC = np.linspace(0.0, 1.0, B, dtype=np.float32)
_SIGMA = (np.mean(np.diff(_BC)) * np.float32(0.5)).astype(np.float32)
_PRETERM = (np.float32(1.0) / (np.float32(2.0) * _SIGMA * _SIGMA)).astype(np.float32)
_CCOMB = float(np.sqrt(np.pi / 2.0))   # infinite Gaussian-comb sum

# consts tile layout (f32, [P, 48]):
#   cols 0..34  : bin centers for blocks 0..34 (33 -> -1/31, 34 -> 32/31)
#   cols 35..38 : -c_0 .. -c_{NACT-1}  (ACT Square biases)
#   col  39     : C = sqrt(pi/2)
#   col  40     : -1.0
#   col  41     : -preterm
NCONST = 48


def _make_consts():
    c = np.zeros((P, NCONST), np.float32)
    cen = np.zeros(NBLK, np.float32)
    cen[0:B] = _BC
    cen[32] = 0.0
    cen[33] = -1.0 / 31.0
    cen[34] = 32.0 / 31.0
    c[:, 0:NBLK] = cen[None, :]
    c[:, 35 : 35 + NACT] = -_BC[None, 0:NACT]
    c[:, 39] = _CCOMB
    c[:, 40] = -1.0
    c[:, 41] = -float(_PRETERM)
    return c


_CACHE = {}


def _build_nc():
    from contextlib import ExitStack
    from concourse import bass, mybir

    f32 = mybir.dt.float32
    f16 = mybir.dt.float16
    AF = mybir.ActivationFunctionType
    AL = mybir.AluOpType

    nc = bass.Bass()
    a_d = nc.dram_tensor("a", [P, V], f32, kind="ExternalInput")
    b_d = nc.dram_tensor("b", [P, V], f32, kind="ExternalInput")
    c_d = nc.dram_tensor("consts", [P, NCONST], f32, kind="ExternalInput")
    out_d = nc.dram_tensor("stats", [S, S], f32, kind="ExternalOutput")

    with ExitStack() as ctx:
        e = ctx.enter_context
        xa = e(nc.sbuf_tensor("xa", [P, V], f32))
        xb = e(nc.sbuf_tensor("xb", [P, V], f32))
        x16a = e(nc.sbuf_tensor("x16a", [P, V], f16))
        x16b = e(nc.sbuf_tensor("x16b", [P, V], f16))
        cst = e(nc.sbuf_tensor("cst", [P, NCONST], f32))
        abuf = [e(nc.sbuf_tensor(f"abuf{k}", [P, NBLK * VC], f16)) for k in range(2)]
        bbuf = [e(nc.sbuf_tensor(f"bbuf{k}", [P, NBLK * VC], f16)) for k in range(2)]
        rsa = [e(nc.sbuf_tensor(f"rsa{k}", [P, VC], f16)) for k in range(2)]
        s1a = e(nc.sbuf_tensor("s1a", [P, VC], f16))
        s1b = e(nc.sbuf_tensor("s1b", [P, VC], f16))
        lnsa = e(nc.sbuf_tensor("lnsa", [P, VC], f32))
        lnsb = e(nc.sbuf_tensor("lnsb", [P, VC], f32))
        stats_sb = e(nc.sbuf_tensor("stats_sb", [S, S], f32))
        acc = e(nc.psum_tensor("acc", [S, S], f32))

        s_dma = e(nc.semaphore("s_dma"))
        s_cast = e(nc.semaphore("s_cast"))
        s_ones = e(nc.semaphore("s_ones"))
        s_sqa = e(nc.semaphore("s_sqa"))
        s_sqb = e(nc.semaphore("s_sqb"))
        s_expa = e(nc.semaphore("s_expa"))
        s_expb = e(nc.semaphore("s_expb"))
        s_s1a = e(nc.semaphore("s_s1a"))
        s_s1b = e(nc.semaphore("s_s1b"))
        s_ra = e(nc.semaphore("s_ra"))
        s_rb = e(nc.semaphore("s_rb"))
        s_mula = e(nc.semaphore("s_mula"))
        s_pe = e(nc.semaphore("s_pe"))
        s_done = e(nc.semaphore("s_done"))
        s_out = e(nc.semaphore("s_out"))
        block = e(nc.Block())

        def bl(buf, i, n=1):
            return buf[:, i * VC : (i + n) * VC]

        def xs(x16, c):
            return x16[:, c * VC : (c + 1) * VC]

        # DVE-computed bin blocks: NACT..31 plus virtual 33, 34
        dve_bins = list(range(NACT, B)) + [33, 34]

        @block.sync
        def _(sync):
            sync.dma_start(cst[:, :], c_d[:, :]).then_inc(s_dma, 16)
            sync.dma_start(xa[:, :], a_d[:, :]).then_inc(s_dma, 16)
            sync.dma_start(xb[:, :], b_d[:, :]).then_inc(s_dma, 16)

        @block.vector
        def _(vector):
            vector.wait_ge(s_dma, 48)
            vector.tensor_copy(x16a[:, :], xa[:, :])
            vector.tensor_copy(x16b[:, :], xb[:, :]).then_inc(s_cast, 1)
            for c in range(NCHUNK):
                k = c % 2
                if c >= 2:
                    vector.wait_ge(s_pe, c - 1)
                for buf, x16, s_sq in (
                    (abuf[k], x16a, s_sqa),
                    (bbuf[k], x16b, s_sqb),
                ):
                    for i in dve_bins:
                        vector.tensor_scalar(
                            bl(buf, i), xs(x16, c), cst[:, i : i + 1], None,
                            AL.subtract,
                        )
                    r1 = bl(buf, NACT, B - NACT)           # blocks NACT..31
                    vector.tensor_mul(r1, r1, r1)
                    r2 = bl(buf, 33, 2)                    # virtual blocks
                    vector.tensor_mul(r2, r2, r2).then_inc(s_sq, 1)

                vector.wait_ge(s_expa, c + 1)
                vector.tensor_add(s1a[:, :], bl(abuf[k], 33), bl(abuf[k], 34))
                vector.engine_nop().then_inc(s_s1a, 1)
                vector.wait_ge(s_expb, c + 1)
                vector.tensor_add(s1b[:, :], bl(bbuf[k], 33), bl(bbuf[k], 34))
                vector.engine_nop().then_inc(s_s1b, 1)

                vector.wait_ge(s_ra, c + 1)
                av = bl(abuf[k], 0, B).rearrange("p (i v) -> p i v", v=VC)
                vector.tensor_mul(
                    av,
                    av,
                    rsa[k][:, :]
                    .rearrange("p (o v) -> p o v", o=1)
                    .broadcast_to([P, B, VC]),
                ).then_inc(s_mula, 1)

            vector.wait_ge(s_pe, NCHUNK)
            vector.tensor_copy(stats_sb[:, :], acc[:, :]).then_inc(s_done, 1)

        @block.scalar
        def _(scalar):
            scalar.wait_ge(s_cast, 1)
            for c in range(NCHUNK):
                k = c % 2
                if c >= 2:
                    scalar.wait_ge(s_pe, c - 1)
                for buf, x16, s_sq, s_exp in (
                    (abuf[k], x16a, s_sqa, s_expa),
                    (bbuf[k], x16b, s_sqb, s_expb),
                ):
                    for i in range(NACT):
                        scalar.activation(
                            bl(buf, i), xs(x16, c), AF.Square,
                            bias=cst[:, 35 + i : 36 + i],
                        )
                    scalar.wait_ge(s_sq, c + 1)
                    r1 = bl(buf, 0, B)
                    scalar.activation(r1, r1, AF.Exp, scale=cst[:, 41:42])
                    r2 = bl(buf, 33, 2)
                    scalar.activation(r2, r2, AF.Exp, scale=cst[:, 41:42]).then_inc(
                        s_exp, 1
                    )
                # 1/S via exp(-ln(C - s1))
                scalar.wait_ge(s_s1a, c + 1)
                scalar.activation(
                    lnsa[:, :], s1a[:, :], AF.Ln,
                    bias=cst[:, 39:40], scale=cst[:, 40:41],
                )
                scalar.activation(
                    rsa[k][:, :], lnsa[:, :], AF.Exp, scale=cst[:, 40:41]
                ).then_inc(s_ra, 1)
                scalar.wait_ge(s_s1b, c + 1)
                scalar.activation(
                    lnsb[:, :], s1b[:, :], AF.Ln,
                    bias=cst[:, 39:40], scale=cst[:, 40:41],
                )
                scalar.activation(
                    bl(abuf[k], 32), lnsb[:, :], AF.Exp, scale=cst[:, 40:41]
                ).then_inc(s_rb, 1)

        @block.tensor
        def _(tensor):
            tensor.wait_ge(s_ones, 2)
            for c in range(NCHUNK):
                k = c % 2
                tensor.wait_ge(s_mula, c + 1)
                tensor.wait_ge(s_rb, c + 1)
                lv = abuf[k][:, :].rearrange("p (i v) -> p i v", v=VC)
                rv = bbuf[k][:, :].rearrange("p (i v) -> p i v", v=VC)
                for v in range(VC):
                    first = c == 0 and v == 0
                    last = c == NCHUNK - 1 and v == VC - 1
                    mm = tensor.matmul(
                        acc[:, :],
                        lv[:, 0:S, v : v + 1],
                        rv[:, 0:S, v : v + 1],
                        start=first,
                        stop=last,
                    )
                    if v == VC - 1:
                        mm.then_inc(s_pe, 1)

        @block.gpsimd
        def _(gpsimd):
            for k in range(2):
                gpsimd.memset(bl(bbuf[k], 32), 1.0).then_inc(s_ones, 1)
            gpsimd.wait_ge(s_done, 1)
            gpsimd.dma_start(out_d[:, :], stats_sb[:, :]).then_inc(s_out, 16)
            gpsimd.wait_ge(s_out, 16)

    return nc


def _get_nc():
    if "nc" not in _CACHE:
        _CACHE["nc"] = _build_nc()
    return _CACHE["nc"]


def run_device(a_flat, b_flat, trace=False):
    """Run the per-core bass kernel on 8 cores; returns (stats_sum, results)."""
    from concourse.bass_utils import run_bass_kernel_spmd

    nc = _get_nc()
    consts = _make_consts()
    a3 = a_flat.reshape(NCORES, P, V)
    b3 = b_flat.reshape(NCORES, P, V)
    in_maps = [
        {"a": np.ascontiguousarray(a3[i]), "b": np.ascontiguousarray(b3[i]),
         "consts": consts}
        for i in range(NCORES)
    ]
    kw = {}
    if trace:
        kw.update(trace=True, trace_cores=[0])
    res = run_bass_kernel_spmd(nc, in_maps, list(range(NCORES)), **kw)
    stats = np.zeros((S, S), np.float64)
    for r in res.results:
        stats += np.asarray(r["stats"], np.float64)
    return stats, res


def finish(stats):
    n = float(NVOX_TOTAL)
    pab = stats[0:B, 0:B] / n
    pa = stats[0:B, B] / n
    pb = stats[B, 0:B] / n
    eps = 1.4e-45
    papb = np.outer(pa, pb) + eps
    mi = np.sum(pab * np.log(pab / papb + eps))
    return np.array([-mi], dtype=np.float32)


def kernel(actual, target):
    a = np.clip(np.asarray(actual, np.float32).reshape(-1), 0.0, 1.0)
    b = np.clip(np.asarray(target, np.float32).reshape(-1), 0.0, 1.0)
    stats, _ = run_device(a, b)
    return finish(stats)
